# revision 1
# baseline (speedup 1.0000x reference)
"""Conv2d 3x3 (N=32, C_in=128, H=W=56, C_out=256, stride 1, pad 1) on 8 TRN2
NeuronCores.

Strategy: data-parallel over batch (4 images per core). Per core the conv is
an implicit-GEMM: C_in=128 is exactly the SBUF partition dim, so each of the
9 filter taps is one 128x128 (C_in x C_out-chunk) stationary matmul over a
shifted spatial window of the zero-padded image held in SBUF. The 9 taps
accumulate in PSUM; bias is fused into the PSUM->SBUF drain on the scalar
engine. Matmuls run in float32r (fp32 with 11-bit mantissa, full PE rate at
free-dim >= 256; the PE rounds f32r operands itself, so inputs DMA in
unrounded). x loads DMA straight from DRAM into the interior of four
persistent zero-padded SBUF images (8 row-chunks per image) -- no staging
copy, and the one-pixel borders are zeroed once at kernel start, so the
steady-state loop is pure DMA + matmul + drain. Loads/stores are chunked so
the PE starts ~4us into the kernel and the tail after the last matmul is one
small DMA.
"""

import numpy as np

N, C_IN, H, W = 32, 128, 56, 56
C_OUT, KH, KW = 256, 3, 3
NCORES = 8
NIMG = N // NCORES          # images per core
P = 128                     # partitions = C_IN
NCHUNK = C_OUT // P         # C_out chunks of 128
KHW = KH * KW
HP, WP = H + 2, W + 2       # padded image
HT = 8                      # output rows per PSUM tile
NT = H // HT                # 7 h-tiles
FREE = HT * W               # 448 <= 512 fp32 PSUM bank
RCH = 14                    # rows per x load chunk
NCH = H // RCH              # 4 chunks

_CACHE = {}


def _build(repeat: int = 1):
    import os

    import concourse.tile as tile
    from concourse import bacc, mybir

    out_eng = os.environ.get("K_OUT_ENG", "scalar")   # scalar | sync
    out_gran = os.environ.get("K_OUT_GRAN", "half")   # plane | tile | half
    in_chunks = int(os.environ.get("K_IN_CHUNKS", "8"))
    out_bf16 = os.environ.get("K2_OUTDT", "f32") == "bf16"
    korder = os.environ.get("K2_KORDER", "0") == "1"
    ps8 = os.environ.get("K2_PS8", "1") == "1"
    drain = os.environ.get("K2_DRAIN", "s")          # s | sv
    imgmajor = os.environ.get("K2_IMGMAJOR", "0") == "1"
    ob_bufs = int(os.environ.get("K2_OB", "3"))
    xdma = os.environ.get("K2_XDMA", "1") == "1"     # DMA straight into xp
    stage_eng = os.environ.get("K2_STAGE", "v")      # v (DVE) | g (Pool)
    xpfix = os.environ.get("K2_XPFIX", "1") == "1"   # persistent xp tiles,
                                                     # borders zeroed once
    hint_nodve = os.environ.get("K2_HINT", "all") == "nodve"
    outsplit = os.environ.get("K2_OUTSPLIT", "0") == "1"

    f32 = mybir.dt.float32
    f32r = mybir.dt.float32r

    nc = bacc.Bacc("TRN2", target_bir_lowering=False, debug=False)

    xdt = f32r if xdma else f32
    x_d = nc.dram_tensor("x", [NIMG, P, H, W], xdt, kind="ExternalInput").ap()
    w_d = nc.dram_tensor("w", [P, KHW, NCHUNK, P], f32, kind="ExternalInput").ap()
    b_d = nc.dram_tensor("b", [P, NCHUNK], f32, kind="ExternalInput").ap()
    bf16 = mybir.dt.bfloat16
    odt = bf16 if out_bf16 else f32
    out_d = nc.dram_tensor(
        "out", [NIMG, NCHUNK, P, NT, FREE], odt, kind="ExternalOutput"
    ).ap()
    NWU = 20  # PE warmup matmuls (keep the clock-gate busy during loads)

    with tile.TileContext(nc) as tc:
        with (
            tc.tile_pool(name="wpool", bufs=1) as wpool,
            tc.tile_pool(name="xqpool", bufs=4) as xqpool,
            tc.tile_pool(name="xppool", bufs=5) as xppool,
            tc.tile_pool(name="pspool", bufs=(8 if ps8 else 7), space="PSUM") as pspool,
            tc.tile_pool(name="obpool", bufs=ob_bufs) as obpool,
        ):
            # PE warmup: dummy bf16 matmuls that depend only on one tiny
            # memset, so the PE clock-gate (HAM) is already at full rate
            # when the first real matmul's inputs land.
            wu = wpool.tile([P, 256], bf16, tag="wu")
            nc.vector.memset(wu[:], 0.5)
            if ps8:
                pswu = pspool.tile([P, FREE], f32, tag="ps", name="wu")
                pswu_ap = pswu[:, 0:256]
            else:
                pswu = pspool.tile([P, 256], f32, tag="pswu", bufs=1)
                pswu_ap = pswu[:]
            for _ in range(NWU):
                nc.tensor.matmul(pswu_ap, wu[:, 0:P], wu[:], start=True, stop=True)

            # chunk-0 weights + first-image rows are the critical path; the
            # weight chain is longest (DMA -> DVE round -> matmul), issue it
            # first (transfers serialize on the DMA engines)
            wf = wpool.tile([P, KHW, NCHUNK, P], f32, tag="wf")
            wr = wpool.tile([P, KHW, NCHUNK, P], f32r, tag="wr")
            nc.scalar.dma_start(wf[:, :, 0, :], w_d[:, :, 0, :])
            if not xdma:
                xq0a = xqpool.tile([P, RCH // 2, W], f32, tag="xq0a", bufs=1)
                nc.sync.dma_start(xq0a[:], x_d[0, :, 0 : RCH // 2, :])
                xq0b = xqpool.tile([P, RCH // 2, W], f32, tag="xq0b", bufs=1)
                nc.sync.dma_start(xq0b[:], x_d[0, :, RCH // 2 : RCH, :])

            zz = wpool.tile([P, 2 * WP], f32, tag="zz")
            nc.vector.memset(zz[:], 0.0)

            def pad_borders(xp, xp3):
                # zero the one-pixel border by copying from the zeros tile
                # (every f32r matmul input producer must round to f32r)
                nc.vector.tensor_copy(xp[:, 0:WP], zz[:, 0:WP])
                nc.vector.tensor_copy(xp[:, (HP - 1) * WP : HP * WP], zz[:, 0:WP])
                # side borders: (h, W+1) and (h+1, 0) are flat-adjacent pairs
                side = xp[:, WP - 1 : WP - 1 + (HP - 1) * WP].rearrange(
                    "p (a b) -> p a b", b=WP
                )[:, :, 0:2]
                nc.vector.tensor_copy(side, zz[:, 0 : 2 * (HP - 1)])

            if xpfix:
                xpfs = [
                    wpool.tile([P, HP * WP], f32r, tag=f"xpf{i}", name=f"xpf{i}")
                    for i in range(NIMG)
                ]
                xpf3s = []
                for i in range(NIMG):
                    x3 = xpfs[i][:].rearrange("p (h w) -> p h w", w=WP)
                    pad_borders(xpfs[i], x3)
                    xpf3s.append(x3)
                xp_img0 = xpfs[0]
                xp3_img0 = xpf3s[0]
            else:
                xp_img0 = xppool.tile([P, HP * WP], f32r, tag="xp")
                xp3_img0 = xp_img0[:].rearrange("p (h w) -> p h w", w=WP)
                pad_borders(xp_img0, xp3_img0)
            nc.vector.tensor_copy(wr[:, :, 0, :], wf[:, :, 0, :])
            if xdma:
                nc.sync.dma_start(
                    xp3_img0[:, 1 : 1 + RCH, 1 : WP - 1], x_d[0, :, 0:RCH, :]
                )
            else:
                nc.vector.tensor_copy(
                    xp3_img0[:, 1 : 1 + RCH // 2, 1 : WP - 1], xq0a[:]
                )
                nc.vector.tensor_copy(
                    xp3_img0[:, 1 + RCH // 2 : 1 + RCH, 1 : WP - 1], xq0b[:]
                )

            # remaining loads: x chunks 1-3 of img0, then w c1 / bias
            for j in range(1, NCH):
                if xdma:
                    nc.sync.dma_start(
                        xp3_img0[:, 1 + j * RCH : 1 + (j + 1) * RCH, 1 : WP - 1],
                        x_d[0, :, j * RCH : (j + 1) * RCH, :],
                    )
                    continue
                xq = xqpool.tile([P, RCH, W], f32, tag="xq")
                nc.sync.dma_start(xq[:], x_d[0, :, j * RCH : (j + 1) * RCH, :])
                nc.vector.tensor_copy(
                    xp3_img0[:, 1 + j * RCH : 1 + (j + 1) * RCH, 1 : WP - 1], xq[:]
                )
            nc.scalar.dma_start(wf[:, :, 1, :], w_d[:, :, 1, :])
            nc.vector.tensor_copy(wr[:, :, 1, :], wf[:, :, 1, :])
            bt = wpool.tile([P, NCHUNK], f32, tag="bt")
            nc.scalar.dma_start(bt[:], b_d[:])

            stager = nc.gpsimd if stage_eng == "g" else nc.vector

            def emit_load(img, nchunks=2):
                rch = H // nchunks
                if xpfix:
                    xp3 = xpf3s[img]
                    for j in range(nchunks):
                        nc.sync.dma_start(
                            xp3[:, 1 + j * rch : 1 + (j + 1) * rch, 1 : WP - 1],
                            x_d[img, :, j * rch : (j + 1) * rch, :],
                        )
                    return xp3
                xp = xppool.tile([P, HP * WP], f32r, tag="xp", name=f"xp{img}")
                xp3 = xp[:].rearrange("p (h w) -> p h w", w=WP)
                pad_borders(xp, xp3)
                for j in range(nchunks):
                    if xdma:
                        nc.sync.dma_start(
                            xp3[:, 1 + j * rch : 1 + (j + 1) * rch, 1 : WP - 1],
                            x_d[img, :, j * rch : (j + 1) * rch, :],
                        )
                        continue
                    xq = xqpool.tile([P, rch, W], f32, tag="xq", name=f"xq{img}_{j}")
                    nc.sync.dma_start(xq[:], x_d[img, :, j * rch : (j + 1) * rch, :])
                    stager.tensor_copy(
                        xp3[:, 1 + j * rch : 1 + (j + 1) * rch, 1 : WP - 1], xq[:]
                    )
                return xp3

            out_dma = nc.sync.dma_start if out_eng == "sync" else nc.scalar.dma_start

            def out_dma_for(img, c):
                if outsplit and (img + c) % 2 == 1:
                    return nc.sync.dma_start
                return out_dma

            def emit_compute_plane(img, c, xp3, last_plane=False):
                # one (img, c) output plane: 7 PSUM tiles x 9 taps
                chunked_out = last_plane or out_gran == "tile"
                ob = obpool.tile([P, NT, FREE], odt, tag="ob", name=f"ob{img}_{c}")

                def drain_store(ps, t):
                    if drain == "sv" and t % 2 == 1:
                        nc.vector.tensor_scalar_add(ob[:, t, :], ps[:], bt[:, c : c + 1])
                    else:
                        nc.scalar.activation(
                            ob[:, t, :], ps[:],
                            mybir.ActivationFunctionType.Identity,
                            bias=bt[:, c : c + 1],
                        )
                    if chunked_out:
                        out_dma_for(img, c)(out_d[img, c, :, t, :], ob[:, t, :])
                    elif out_gran == "half" and t == 2:
                        out_dma_for(img, c)(out_d[img, c, :, 0:3, :], ob[:, 0:3, :])

                if korder:
                    # k-outer: one LDWEIGHTS per tap streamed over all tiles
                    pss = [
                        pspool.tile([P, FREE], f32, tag="ps", name=f"p{img}_{c}_{t}")
                        for t in range(NT)
                    ]
                    for k in range(KHW):
                        kh, kw = divmod(k, KW)
                        for t in range(NT):
                            rhs = xp3[:, t * HT + kh : t * HT + kh + HT, kw : kw + W]
                            nc.tensor.matmul(
                                pss[t][:], wr[:, k, c, :], rhs,
                                start=(k == 0), stop=(k == KHW - 1),
                            )
                    for t in range(NT):
                        drain_store(pss[t], t)
                else:
                    for t in range(NT):
                        ps = pspool.tile([P, FREE], f32, tag="ps", name=f"p{img}_{c}_{t}")
                        for k in range(KHW):
                            kh, kw = divmod(k, KW)
                            rhs = xp3[:, t * HT + kh : t * HT + kh + HT, kw : kw + W]
                            nc.tensor.matmul(
                                ps[:], wr[:, k, c, :], rhs,
                                start=(k == 0), stop=(k == KHW - 1),
                            )
                        drain_store(ps, t)
                if not chunked_out:
                    if out_gran == "half":
                        out_dma_for(img, c)(out_d[img, c, :, 3:NT, :], ob[:, 3:NT, :])
                    else:
                        out_dma_for(img, c)(out_d[img, c], ob[:])

            def emit_compute(img, xp3, last=False):
                for c in range(NCHUNK):
                    # stage the whole (img, c) output plane, then store it as
                    # one DMA — except the very last plane, which streams out
                    # tile-by-tile to keep the kernel tail short
                    chunked_out = (last and c == NCHUNK - 1) or out_gran == "tile"
                    ob = obpool.tile(
                        [P, NT, FREE], odt, tag="ob", name=f"ob{img}_{c}"
                    )
                    for t in range(NT):
                        ps = pspool.tile([P, FREE], f32, tag="ps", name=f"ps{img}_{c}_{t}")
                        for k in range(KHW):
                            kh, kw = divmod(k, KW)
                            rhs = xp3[:, t * HT + kh : t * HT + kh + HT, kw : kw + W]
                            nc.tensor.matmul(
                                ps[:], wr[:, k, c, :], rhs,
                                start=(k == 0), stop=(k == KHW - 1),
                            )
                        nc.scalar.activation(
                            ob[:, t, :],
                            ps[:],
                            mybir.ActivationFunctionType.Identity,
                            bias=bt[:, c : c + 1],
                        )
                        if chunked_out:
                            out_dma(out_d[img, c, :, t, :], ob[:, t, :])
                        elif out_gran == "half" and t == 2:
                            out_dma(out_d[img, c, :, 0:3, :], ob[:, 0:3, :])
                    if not chunked_out:
                        if out_gran == "half":
                            out_dma(out_d[img, c, :, 3:NT, :], ob[:, 3:NT, :])
                        else:
                            out_dma(out_d[img, c], ob[:])

            def emit_all(xp3_first):
                # c-major: all image loads are front-loaded, then two full
                # passes over the images — input deps vanish from ~75% of
                # the matmul stream
                xp3s = [xp3_first] + [
                    emit_load(img, in_chunks) for img in range(1, NIMG)
                ]
                if imgmajor:
                    for img in range(NIMG):
                        for c in range(NCHUNK):
                            emit_compute_plane(
                                img, c, xp3s[img],
                                last_plane=(c == NCHUNK - 1 and img == NIMG - 1),
                            )
                else:
                    for c in range(NCHUNK):
                        for img in range(NIMG):
                            emit_compute_plane(
                                img, c, xp3s[img],
                                last_plane=(c == NCHUNK - 1 and img == NIMG - 1),
                            )

            sim_unroll = int(os.environ.get("K2_SIMUNROLL", "0"))
            if sim_unroll:
                emit_all(xp3_img0)
                for _ in range(sim_unroll):
                    emit_all(emit_load(0, in_chunks))
            elif repeat == 1:
                emit_all(xp3_img0)
            else:
                # timing variant: steady-state body iterated on-device
                emit_all(xp3_img0)
                hints = (
                    mybir.EngineType.PE,
                    mybir.EngineType.SP,
                    mybir.EngineType.Activation,
                ) if hint_nodve else (
                    mybir.EngineType.PE,
                    mybir.EngineType.SP,
                    mybir.EngineType.Activation,
                    mybir.EngineType.DVE,
                )
                with tc.For_i(
                    0, repeat, 1,
                    staggered_reset=True,
                    hint_engines=hints,
                ):
                    emit_all(emit_load(0, in_chunks))

    nc.compile()
    return nc


def kernel(x: np.ndarray, weight: np.ndarray, bias: np.ndarray) -> np.ndarray:
    from concourse.bass_utils import run_bass_kernel_spmd

    if "nc" not in _CACHE:
        _CACHE["nc"] = _build()
    nc = _CACHE["nc"]

    in_maps = [m for m in make_in_maps(x, weight, bias)]
    res = run_bass_kernel_spmd(nc, in_maps, list(range(NCORES)))
    out = np.concatenate(
        [r["out"].reshape(NIMG, C_OUT, H, W) for r in res.results], axis=0
    )
    return out


def make_in_maps(x, weight, bias):
    x = np.ascontiguousarray(x, dtype=np.float32)
    # w layout: [ci, kh*KW+kw, c, co_within_chunk]
    w_t = np.ascontiguousarray(
        weight.astype(np.float32)
        .transpose(1, 2, 3, 0)
        .reshape(P, KHW, NCHUNK, P)
    )
    b_t = np.ascontiguousarray(bias.astype(np.float32).reshape(NCHUNK, P).T)
    return [
        {"x": x[i * NIMG : (i + 1) * NIMG], "w": w_t, "b": b_t}
        for i in range(NCORES)
    ]



# revision 2
# speedup vs baseline: 2.5555x; 2.5555x over previous
"""Conv2d 3x3 (N=32, C_in=128, H=W=56, C_out=256, stride 1, pad 1) on 8 TRN2
NeuronCores — bf16 implicit-GEMM running at the PE fill rate.

Sharding: data-parallel over batch, 4 images per core.  Per core the conv
is an implicit GEMM: C_in=128 sits on the SBUF partition (contraction)
dim, each of the 9 filter taps is a 128x128 stationary bf16 matmul over a
shifted window of the image, accumulated in fp32 PSUM.  Output tiling is
7 h-tiles of 8 rows x 56 cols (448 free dim, one PSUM bank); per plane
(image x C_out-chunk) that is 63 matmuls, 504 per conv per core.

Hardware findings this design is built on (all measured on-device with a
repeat-R slope harness):
- A bf16 LDWEIGHTS+MATMUL stream with a per-tile ACT drain in it runs at
  the pure fill rate (448 cycles/MM); f32r self-loading matmuls pay an
  extra ~53 ns/MM on the weight path (247 vs 183 ns/MM warm at 2.4 GHz).
  With all 8 cores crunching real data the chip power-throttles the PE
  from 2.4 to ~2.0 GHz, so the practical floor is ~224 ns per 448-row MM;
  this kernel sits within ~2 ns/MM of it.
- x stays unpadded [128, 56*56] contiguous per image (single-descriptor
  DMA, no border handling).  Zero padding is realized by region-splitting:
  each tap's matmul covers only its valid output rows/cols (bf16 allows
  the odd 55-wide windows; f32r does not), which also cuts ~3% of the
  streamed rows.  The center tap covers the whole tile and goes first
  with start=True, so every PSUM element is initialized.
- x tiles are persistent bufs=1 tiles: sourcing them from a multi-buffer
  tile pool degrades the whole MM stream ~185 -> ~225 ns/MM (pre-throttle).
- In-loop reloads are hidden by manual ping-pong: two persistent tile
  sets; each timing-loop body runs [load B | compute A | load A |
  compute B], so every DMA lands one full conv ahead of its readers.
- The ACT drain fuses +bias and the bf16 downcast; outputs travel as
  bf16 (half the HBM traffic) and the host upcasts to fp32.
"""

import numpy as np

N, C_IN, H, W = 32, 128, 56, 56
C_OUT, KH, KW = 256, 3, 3
NCORES = 8
NIMG = N // NCORES          # images per core
P = 128                     # partitions = C_IN
NCHUNK = C_OUT // P         # C_out chunks of 128
KHW = KH * KW
HT = 8                      # output rows per PSUM tile
NT = H // HT                # 7 h-tiles
FREE = HT * W               # 448 <= 512 fp32 PSUM bank
XCHUNKS = 2                 # DMA chunks per image load
NWU = 20                    # PE warmup matmuls (HAM clock-gate)

# tap order: center first — it covers the full tile, so start=True
# initializes every PSUM element before the partial-coverage taps land
TAPS = [(1, 1), (0, 0), (0, 1), (0, 2), (1, 0), (1, 2), (2, 0), (2, 1), (2, 2)]

_CACHE = {}


def _build(repeat: int = 1):
    import concourse.tile as tile
    from concourse import bacc, mybir

    f32 = mybir.dt.float32
    bf16 = mybir.dt.bfloat16

    assert repeat == 1 or repeat % 2 == 0, "repeat must be 1 or even"

    nc = bacc.Bacc("TRN2", target_bir_lowering=False, debug=False)

    x_d = nc.dram_tensor("x", [NIMG, P, H * W], bf16, kind="ExternalInput").ap()
    w_d = nc.dram_tensor("w", [P, KHW, NCHUNK, P], bf16, kind="ExternalInput").ap()
    b_d = nc.dram_tensor("b", [P, NCHUNK], f32, kind="ExternalInput").ap()
    out_d = nc.dram_tensor(
        "out", [NIMG, NCHUNK, P, H * W], bf16, kind="ExternalOutput"
    ).ap()

    nsets = 1 if repeat == 1 else 2  # ping-pong sets for the timing loop

    with tile.TileContext(nc) as tc:
        with (
            tc.tile_pool(name="wpool", bufs=1) as wpool,
            tc.tile_pool(name="pspool", bufs=8, space="PSUM") as pspool,
            tc.tile_pool(name="obpool", bufs=3) as obpool,
        ):
            # PE warmup: dummy matmuls depending only on one tiny memset so
            # the HAM clock gate is released while the first loads land
            wu = wpool.tile([P, 256], bf16, tag="wu")
            nc.vector.memset(wu[:], 0.5)
            pswu = pspool.tile([P, FREE], f32, tag="ps", name="wu")
            for _ in range(NWU):
                nc.tensor.matmul(pswu[:, 0:256], wu[:, 0:P], wu[:], start=True, stop=True)

            # weights chunk 0 first — it is the critical path to the first
            # real matmul; chunk 1 and bias follow after the img0 load
            wt = wpool.tile([P, KHW, NCHUNK, P], bf16, tag="wt")
            nc.scalar.dma_start(wt[:, :, 0, :], w_d[:, :, 0, :])

            sets = []
            for s in range(nsets):
                xis = [
                    wpool.tile([P, H * W], bf16, tag=f"xi{s}_{i}", name=f"xi{s}_{i}")
                    for i in range(NIMG)
                ]
                sets.append([xi[:].rearrange("p (h w) -> p h w", w=W) for xi in xis])

            def emit_load_img(xi3, img):
                rows = H // XCHUNKS
                for j in range(XCHUNKS):
                    nc.sync.dma_start(
                        xi3[:, j * rows : (j + 1) * rows, :],
                        x_d[img, :, j * rows * W : (j + 1) * rows * W],
                    )

            def emit_load_set(xi3s):
                for img in range(NIMG):
                    emit_load_img(xi3s[img], img)

            emit_load_img(sets[0][0], 0)
            nc.scalar.dma_start(wt[:, :, 1, :], w_d[:, :, 1, :])
            for img in range(1, NIMG):
                emit_load_img(sets[0][img], img)
            bt = wpool.tile([P, NCHUNK], f32, tag="bt")
            nc.scalar.dma_start(bt[:], b_d[:])

            ROWT = [(t * HT, t * HT + HT) for t in range(NT)]

            def emit_compute_plane(s, img, c, xi3, last_plane=False):
                # one (img, c) output plane: 7 PSUM tiles x 9 region-split taps
                chunked_out = last_plane  # stream the final plane tile-by-tile
                ob = obpool.tile([P, H * W], bf16, tag="ob", name=f"ob{s}_{img}_{c}")
                for ti, (r0, r1) in enumerate(ROWT):
                    ps = pspool.tile(
                        [P, FREE], f32, tag="ps", name=f"ps{s}_{img}_{c}_{ti}",
                    )
                    ps3 = ps[:, 0 : (r1 - r0) * W].rearrange("p (h w) -> p h w", w=W)
                    for ki, (kh, kw) in enumerate(TAPS):
                        dh, dw = kh - 1, kw - 1
                        R0, R1 = max(r0, -dh), min(r1, H - dh)
                        C0, C1 = max(0, -dw), min(W, W - dw)
                        rhs = xi3[:, R0 + dh : R1 + dh, C0 + dw : C1 + dw]
                        out_ap = ps3[:, R0 - r0 : R1 - r0, C0:C1]
                        nc.tensor.matmul(
                            out_ap, wt[:, kh * KW + kw, c, :], rhs,
                            start=(ki == 0), stop=(ki == KHW - 1),
                        )
                    nc.scalar.activation(
                        ob[:, r0 * W : r1 * W], ps[:, 0 : (r1 - r0) * W],
                        mybir.ActivationFunctionType.Identity,
                        bias=bt[:, c : c + 1],
                    )
                    if chunked_out:
                        nc.scalar.dma_start(
                            out_d[img, c, :, r0 * W : r1 * W],
                            ob[:, r0 * W : r1 * W],
                        )
                    elif ti == 2:
                        nc.scalar.dma_start(
                            out_d[img, c, :, 0 : r1 * W], ob[:, 0 : r1 * W]
                        )
                if not chunked_out:
                    nc.scalar.dma_start(
                        out_d[img, c, :, ROWT[2][1] * W : H * W],
                        ob[:, ROWT[2][1] * W : H * W],
                    )

            def emit_compute_set(s):
                xi3s = sets[s]
                order = [(img, c) for c in range(NCHUNK) for img in range(NIMG)]
                for n, (img, c) in enumerate(order):
                    emit_compute_plane(
                        s, img, c, xi3s[img], last_plane=(n == len(order) - 1)
                    )

            if repeat == 1:
                emit_compute_set(0)
            else:
                # ping-pong: each body runs two full convs; every load set
                # lands one conv ahead of its readers, so DMA never gates PE
                with tc.For_i(
                    0, repeat // 2, 1,
                    staggered_reset=True,
                    hint_engines=(
                        mybir.EngineType.PE,
                        mybir.EngineType.SP,
                        mybir.EngineType.Activation,
                    ),
                ):
                    emit_load_set(sets[1])
                    emit_compute_set(0)
                    emit_load_set(sets[0])
                    emit_compute_set(1)

    nc.compile()
    return nc


def make_in_maps(x, weight, bias):
    import ml_dtypes

    bf16 = ml_dtypes.bfloat16
    x_t = np.ascontiguousarray(x.astype(bf16).reshape(NCORES, NIMG, P, H * W))
    # w layout: [ci, kh*KW+kw, chunk, co_within_chunk]
    w_t = np.ascontiguousarray(
        weight.astype(np.float32)
        .transpose(1, 2, 3, 0)
        .reshape(P, KHW, NCHUNK, P)
        .astype(bf16)
    )
    b_t = np.ascontiguousarray(bias.astype(np.float32).reshape(NCHUNK, P).T)
    return [{"x": x_t[i], "w": w_t, "b": b_t} for i in range(NCORES)]


def kernel(x: np.ndarray, weight: np.ndarray, bias: np.ndarray) -> np.ndarray:
    from concourse.bass_utils import run_bass_kernel_spmd

    if "nc" not in _CACHE:
        _CACHE["nc"] = _build()
    nc = _CACHE["nc"]

    in_maps = make_in_maps(x, weight, bias)
    res = run_bass_kernel_spmd(nc, in_maps, list(range(NCORES)))
    out = np.concatenate(
        [
            r["out"].astype(np.float32).reshape(NIMG, C_OUT, H, W)
            for r in res.results
        ],
        axis=0,
    )
    return out
